# revision 40
# baseline (speedup 1.0000x reference)
"""Trainium2 Bass kernel for nn_ExampleBinaryNet (binarized LeNet-style CNN).

Data parallel over 8 NeuronCores, 256 images each. Per core:
  conv1 (3->100, 5x5): im2col to K=75 rows (r = ky*15 + ci*5 + kx), built by
    ONE 4D-AP dma_start per 16-image tile (hi fp16 on the sync HWDGE ring,
    fp8 residual lo on the scalar ring) so the transfer spans all 75 dest
    partitions and engages ~11-16 SDMA engines. Two accumulating matmuls per
    half-image: fp16 hi + plain-fp8 lo (x = fp16(x) + 2^-6 fp8((x-hi)*64)).
  epilogue (hardtanh+maxpool folded): activations are stored centered,
    r = clip(z,-1,1) itself, via r = min(relu(z+1),2)-1; so conv weights
    stay plain signs and biases need no sign-sum corrections.
    Route D images: ACT evicts relu(P+b+1) with x-parity-deinterleaved
    write, then two fp16 tensor_tensor max ops (2x DVE mode) do the 2x2
    maxpool; route A images: DVE tensor_reduce(max) straight from PSUM then
    a tiny ACT. One batched tensor_scalar (min 2, sub 1) per tile finishes.
  conv2 (100->16, 5x5): 25 accumulating tap matmuls, K=100, 4-way PE column
    tiling into ONE shared single-bank PSUM tile; pool2 epilogue runs
    full-partition-span ops covering all 4 groups at once.
  fc1/fc2/fc3: fp16 matmuls (fc1 as 25 accumulating K=16 taps), N=256.
"""

import os
import sys

for _p in ("/opt/trn_rl_repo", "/root/.axon_site/_ro/trn_rl_repo"):
    if os.path.isdir(_p) and _p not in sys.path:
        sys.path.insert(0, _p)

import numpy as np
import ml_dtypes

import concourse.bass as bass
import concourse.tile as tile
from concourse import bacc, mybir
from concourse.bass_utils import run_bass_kernel_spmd

F32 = mybir.dt.float32
FP16 = mybir.dt.float16
FP8 = mybir.dt.float8e4
FP8NP = ml_dtypes.float8_e4m3

NCORES = 8
BPC = 256          # batch per core
NB = 16            # images per batch-tile
NT = BPC // NB     # batch-tiles per core
XPAD = BPC * 1024 + 1024   # flat padded per-channel image stream
# per-image trimmed stream: conv1 only reads offsets 0..895 of each 1024
# (y rows 28..31 are pool margin); host repacks images at 896 stride to cut
# im2col DMA bytes by 12.5%
IMW = 896
XPADT = BPC * IMW + 1024
N_ROUTE_A = 0      # images per tile pooled by DVE straight from PSUM
LO_SCALE = 64.0
# partition base for im2col/weight tiles. Would love 32 to spread the
# im2col DMA over all 16 SDMA engines, but LDWEIGHTS at base 32 may only
# span 32 partitions (BIR verifier), so K=75 weights must sit at base 0.
PBASE = 0


def _build(route_a=N_ROUTE_A, pbase=PBASE, debug=False):
    nc = bacc.Bacc("TRN2", target_bir_lowering=False, debug=False)

    # ---------------- DRAM I/O ----------------
    # xh/xl are host-pre-expanded along ky only: row a = ci*5 + ky holds the
    # channel stream shifted by 32*ky. The kx shifts come from a 3D DMA
    # access pattern [[XPAD,15],[1,5],[1,N]], giving the full 75-row im2col
    # (r = ci*25 + ky*5 + kx) per tile in one dma_start. 75 dest rows ->
    # 15 SDMA engines (5 rows each), while HBM reads stay inside a hot
    # 7.9 MB region (kx re-reads hit open rows).
    xh_d = nc.dram_tensor("xh", [15, XPADT], FP16, kind="ExternalInput")
    xl_d = nc.dram_tensor("xl", [15, XPADT], FP8, kind="ExternalInput")
    w1_d = nc.dram_tensor("w1t", [75, 112], FP16, kind="ExternalInput")
    w1l_d = nc.dram_tensor("w1l", [75, 112], FP8, kind="ExternalInput")
    w2_d = nc.dram_tensor("w2t", [100, 25, 16], FP16, kind="ExternalInput")
    w3_d = nc.dram_tensor("w3t", [16, 25, 120], FP16, kind="ExternalInput")
    w4_d = nc.dram_tensor("w4t", [120, 84], FP16, kind="ExternalInput")
    w5_d = nc.dram_tensor("w5t", [84, 10], FP16, kind="ExternalInput")
    b1p_d = nc.dram_tensor("b1p", [112, 1], F32, kind="ExternalInput")
    b2p_d = nc.dram_tensor("b2p", [112, 1], F32, kind="ExternalInput")
    b3p_d = nc.dram_tensor("b3p", [120, 1], F32, kind="ExternalInput")
    b4p_d = nc.dram_tensor("b4p", [84, 1], F32, kind="ExternalInput")
    b5e_d = nc.dram_tensor("b5e", [10, 1], F32, kind="ExternalInput")
    y_d = nc.dram_tensor("y", [10, BPC], F32, kind="ExternalOutput")
    if debug:
        dbg_r2 = nc.dram_tensor("dbg_r2", [100, NB, 196], FP16,
                                kind="ExternalOutput")
        dbg_r2p = nc.dram_tensor("dbg_r2p", [16, BPC, 25], FP16,
                                 kind="ExternalOutput")

    nA = route_a
    pb = pbase
    NJ = NB // 4  # images per conv2 column group
    # route is assigned per image-PAIR (route-D TT-maxes batch two images);
    # spread route-A pairs across the tile so DVE/ACT load interleaves
    npair = NB // 2
    na_pairs = nA // 2
    a_pairs = (
        {round(i * npair / na_pairs) for i in range(na_pairs)}
        if na_pairs > 0 else set()
    )

    with tile.TileContext(nc) as tc:
        with (
            tc.tile_pool(name="consts", bufs=1) as consts,
            tc.tile_pool(name="im_p", bufs=3) as im_p,
            tc.tile_pool(name="iml_p", bufs=2) as iml_p,
            tc.tile_pool(name="ep_p", bufs=2) as ep_p,
            tc.tile_pool(name="r2_p", bufs=2) as r2_p,
            tc.tile_pool(name="p2_p", bufs=2) as p2_p,
            tc.tile_pool(name="fc_p", bufs=1) as fc_p,
            tc.tile_pool(name="ps1_p", bufs=3, space="PSUM") as ps1_p,
            tc.tile_pool(name="ps2_p", bufs=1, space="PSUM") as ps2_p,
        ):
            # ---------------- constants ----------------
            w1full = consts.tile([pb + 75, 112], FP16, name="w1full")
            w1lfull = consts.tile([pb + 75, 112], FP8, name="w1lfull")
            w1sb = w1full[pb : pb + 75, :]
            w1lsb = w1lfull[pb : pb + 75, :]
            w2sb = consts.tile([100, 25, 16], FP16)
            w3sb = consts.tile([16, 25, 120], FP16)
            w4sb = consts.tile([120, 84], FP16)
            w5sb = consts.tile([84, 10], FP16)
            b1p = consts.tile([112, 1], F32)
            b2p = consts.tile([112, 1], F32)
            b3p = consts.tile([120, 1], F32)
            b4p = consts.tile([84, 1], F32)
            b5e = consts.tile([10, 1], F32)
            r2p = consts.tile([16, BPC, 25], FP16, name="r2p")
            # only conv1's weights/bias gate the first matmul; the rest load
            # behind the first im2col DMA to shorten the startup ramp
            for t_sb, t_d in [(w1sb, w1_d), (w1lsb, w1l_d), (b1p, b1p_d)]:
                nc.sync.dma_start(out=t_sb, in_=t_d[:])

            def load_late_consts():
                for t_sb, t_d in [
                    (w2sb, w2_d), (w3sb, w3_d), (w4sb, w4_d), (w5sb, w5_d),
                    (b2p, b2p_d), (b3p, b3p_d), (b4p, b4p_d), (b5e, b5e_d),
                ]:
                    nc.sync.dma_start(out=t_sb, in_=t_d[:])

            prev = None  # state for conv2 stage of previous batch-tile

            def conv2_block(pv):
                """conv2 + pool2 + fc-input epilogue for one batch-tile."""
                it, r2 = pv
                pg = ps2_p.tile([128, 512], F32, name=f"pg_{it}", tag="pg")
                # r2 viewed as [100, j, g, 14, 14] with local image b = 4j+g
                r2v = r2[:].rearrange("p (j g) (y x) -> p j g y x", g=4, x=14)
                for t in range(25):
                    ky, kx = divmod(t, 5)
                    for g in range(4):
                        rhs = r2v[:, :, g, ky : ky + 10, kx : kx + 10]
                        nc.tensor.matmul(
                            pg[32 * g : 32 * g + 16, 0 : 100 * NJ],
                            w2sb[:, t, :],
                            rhs,
                            start=(t == 0),
                            stop=(t == 24),
                            tile_position=(0, 32 * g),
                        )
                # one full-partition-span contiguous evict: relu(P2+(b2+1));
                # garbage partition strips (16..31 etc) are written but never
                # read downstream.
                ev2 = p2_p.tile([112, NJ, 10, 10], FP16, name=f"ev2_{it}",
                                tag="ev2")
                nc.scalar.activation(
                    out=ev2[:].rearrange("p j y x -> p (j y x)"),
                    in_=pg[0:112, 0 : 100 * NJ],
                    func=mybir.ActivationFunctionType.Relu,
                    bias=b2p[:],
                    scale=1.0,
                )
                # maxpool 2x2: x-pairs (strided, 1x) then y-pairs
                m1p = p2_p.tile([112, NJ, 10, 5], FP16, name=f"m1p_{it}",
                                tag="m1p")
                ev2v = ev2[:].rearrange("p j y (xa xb) -> p j y xa xb", xb=2)
                nc.vector.tensor_tensor(
                    m1p[:].rearrange("p j y xa -> p (j y xa)"),
                    ev2v[:, :, :, :, 0].rearrange("p j y xa -> p (j y xa)"),
                    ev2v[:, :, :, :, 1].rearrange("p j y xa -> p (j y xa)"),
                    mybir.AluOpType.max,
                )
                m2p = p2_p.tile([112, NJ, 5, 5], FP16, name=f"m2p_{it}",
                                tag="m2p")
                m1v = m1p[:].rearrange("p j (ya yb) xa -> p j ya yb xa", yb=2)
                nc.vector.tensor_tensor(
                    m2p[:], m1v[:, :, :, 0, :], m1v[:, :, :, 1, :],
                    mybir.AluOpType.max,
                )
                # r2p = min(m2, 2) - 1  (store h2 in [-1,1]); per-group
                # cross-partition remap into partitions 0..16
                r2pv = r2p[:].rearrange("p (t j g) f -> p t j g f", t=NT, g=4)
                for g in range(4):
                    nc.vector.tensor_scalar(
                        out=r2pv[:, it, :, g, :],
                        in0=m2p[32 * g : 32 * g + 16].rearrange(
                            "p j a b -> p j (a b)"
                        ),
                        scalar1=2.0,
                        scalar2=1.0,
                        op0=mybir.AluOpType.min,
                        op1=mybir.AluOpType.subtract,
                    )

            for it in range(NT):
                # -------- im2col: one rectangular dma_start each ----------
                base = it * NB * IMW
                im = im_p.tile([pb + 75, NB * IMW], FP16, name=f"im_{it}",
                               tag="im")
                iml = iml_p.tile([pb + 75, NB * IMW], FP8, name=f"iml_{it}",
                                 tag="iml")
                src_hi = bass.AP(
                    tensor=xh_d.ap().tensor,
                    offset=base,
                    ap=[[XPADT, 15], [1, 5], [1, NB * IMW]],
                )
                nc.sync.dma_start(out=im[pb : pb + 75, :], in_=src_hi)
                src_lo = bass.AP(
                    tensor=xl_d.ap().tensor,
                    offset=base,
                    ap=[[XPADT, 15], [1, 5], [1, NB * IMW]],
                )
                # same ring as hi: a 75-row DMA already spans ~15 of the 16
                # physical SDMA engines, so a second ring only time-shares
                # them at half rate (measured 13.7 vs 27 GB/s per slot)
                nc.sync.dma_start(out=iml[pb : pb + 75, :], in_=src_lo)
                if it == 0:
                    load_late_consts()

                # ---------------- conv2 of previous tile ----------------
                # emitted BEFORE this tile's conv1 so the 25 taps run dense
                # on the PE and the 4 column-tiled matmuls per tap actually
                # overlap (interleaving conv1 between them serializes all)
                if prev is not None:
                    conv2_block(prev)

                # ---------------- conv1 + pool1 epilogue ----------------
                r2u = ep_p.tile([100, NB, 196], FP16, name=f"r2u_{it}",
                                tag="r2u")
                r2 = r2_p.tile([100, NB, 196], FP16, name=f"r2_{it}", tag="r2")

                for pi in range(npair):
                    # pair the hi (then lo) matmuls of two images so the PE
                    # keeps the same stationary weights across 4 consecutive
                    # matmuls instead of swapping hi/lo every matmul
                    pair = (2 * pi, 2 * pi + 1)
                    ps1s = {}
                    for bb in pair:
                        ps1s[bb] = ps1_p.tile(
                            [112, 2, 512], F32, name=f"ps1_{it}_{bb}",
                            tag="ps1"
                        )
                    for w_sb, i_t, first in ((w1sb, im, True),
                                             (w1lsb, iml, False)):
                        for bb in pair:
                            ib = i_t[
                                pb : pb + 75,
                                bb * IMW : bb * IMW + IMW,
                            ].rearrange("p (y w) -> p y w", w=32)
                            for h in range(2):
                                nc.tensor.matmul(
                                    ps1s[bb][:, h, 0:392],
                                    w_sb,
                                    ib[:, 14 * h : 14 * h + 14, 0:28],
                                    start=first,
                                    stop=not first,
                                )
                    if pi in a_pairs:
                        # route A: DVE maxpool straight from PSUM (per bank),
                        # then ACT relu(. + b1 + 1)
                        for b in pair:
                            praw = ep_p.tile([100, 2, 7, 14], FP16,
                                             name=f"praw_{it}_{b}",
                                             tag="praw")
                            for h in range(2):
                                nc.vector.tensor_reduce(
                                    out=praw[:, h],
                                    in_=ps1s[b][0:100, h, 0:392].rearrange(
                                        "p (y a x b) -> p y x a b",
                                        y=7, a=2, b=2
                                    ),
                                    axis=mybir.AxisListType.XY,
                                    op=mybir.AluOpType.max,
                                )
                            nc.scalar.activation(
                                out=r2u[:, b, :],
                                in_=praw[:].rearrange("p h y x -> p (h y x)"),
                                func=mybir.ActivationFunctionType.Relu,
                                bias=b1p[0:100],
                                scale=1.0,
                            )
                    else:
                        # route D: per-image ACT evict relu(P + b1 + 1) with
                        # x-parity deinterleave via a write AP whose inner
                        # run is 14 contiguous elements; then ONE 2x-mode
                        # TT-max per pool stage covering both images
                        ev1 = ep_p.tile([100, 2, 2, 28, 14], FP16,
                                        name=f"ev1_{it}_{pi}", tag="ev1")
                        for i, b in enumerate(pair):
                            out_ap = ev1[:, i].rearrange(
                                "p xb (h y) xa -> p xb h y xa", h=2
                            )
                            in_ap = ps1s[b][0:100, :, 0:392].rearrange(
                                "p h (y xa xb) -> p xb h y xa", y=14, xb=2
                            )
                            if i == 1 and pi % 2 == 0:
                                # evict every other pair's second image on
                                # DVE so the pair's two evictions overlap
                                # and PSUM slots free sooner
                                nc.vector.tensor_scalar(
                                    out=out_ap, in0=in_ap,
                                    scalar1=b1p[0:100], scalar2=0.0,
                                    op0=mybir.AluOpType.add,
                                    op1=mybir.AluOpType.max,
                                )
                            else:
                                nc.scalar.activation(
                                    out=out_ap, in_=in_ap,
                                    func=mybir.ActivationFunctionType.Relu,
                                    bias=b1p[0:100],
                                    scale=1.0,
                                )
                        m1 = ep_p.tile([100, 2, 28, 14], FP16,
                                       name=f"m1_{it}_{pi}", tag="m1")
                        nc.vector.tensor_tensor(
                            m1[:],
                            ev1[:, :, 0],
                            ev1[:, :, 1],
                            mybir.AluOpType.max,
                        )
                        m1v = m1[:].rearrange(
                            "p i (ya yb) x -> p i ya yb x", yb=2
                        )
                        nc.vector.tensor_tensor(
                            r2u[:, 2 * pi : 2 * pi + 2, :].rearrange(
                                "p i (y x) -> p i y x", x=14
                            ),
                            m1v[:, :, :, 0, :],
                            m1v[:, :, :, 1, :],
                            mybir.AluOpType.max,
                        )

                # batched: r2 = min(r2u, 2) - 1  (store h1 in [-1,1])
                nc.vector.tensor_scalar(
                    out=r2[:].rearrange("p b f -> p (b f)"),
                    in0=r2u[:].rearrange("p b f -> p (b f)"),
                    scalar1=2.0,
                    scalar2=1.0,
                    op0=mybir.AluOpType.min,
                    op1=mybir.AluOpType.subtract,
                )

                if debug and it == 0:
                    nc.sync.dma_start(out=dbg_r2[:], in_=r2[:])

                prev = (it, r2)

            conv2_block(prev)

            # ---------------- fully connected layers ----------------
            if debug:
                nc.sync.dma_start(out=dbg_r2p[:], in_=r2p[:])
            ps3 = ps1_p.tile([120, BPC], F32, name="ps3", tag="ps1")
            for p in range(25):
                nc.tensor.matmul(
                    ps3[:],
                    w3sb[:, p, :],
                    r2p[:, :, p],
                    start=(p == 0),
                    stop=(p == 24),
                )
            u3 = fc_p.tile([120, BPC], F32)
            nc.scalar.activation(
                out=u3[:], in_=ps3[:],
                func=mybir.ActivationFunctionType.Relu,
                bias=b3p[:], scale=1.0,
            )
            r3 = fc_p.tile([120, BPC], FP16)
            nc.vector.tensor_scalar(
                out=r3[:], in0=u3[:], scalar1=2.0, scalar2=1.0,
                op0=mybir.AluOpType.min, op1=mybir.AluOpType.subtract,
            )

            ps4 = ps1_p.tile([84, BPC], F32, name="ps4", tag="ps1")
            nc.tensor.matmul(ps4[:], w4sb[:], r3[:], start=True, stop=True)
            u4 = fc_p.tile([84, BPC], F32)
            nc.scalar.activation(
                out=u4[:], in_=ps4[:],
                func=mybir.ActivationFunctionType.Relu,
                bias=b4p[:], scale=1.0,
            )
            r4 = fc_p.tile([84, BPC], FP16)
            nc.vector.tensor_scalar(
                out=r4[:], in0=u4[:], scalar1=2.0, scalar2=1.0,
                op0=mybir.AluOpType.min, op1=mybir.AluOpType.subtract,
            )

            ps5 = ps1_p.tile([10, BPC], F32, name="ps5", tag="ps1")
            nc.tensor.matmul(ps5[:], w5sb[:], r4[:], start=True, stop=True)
            y_sb = fc_p.tile([10, BPC], F32)
            nc.vector.tensor_scalar_add(y_sb[:], ps5[:], b5e[:])
            nc.sync.dma_start(out=y_d[:], in_=y_sb[:])

    nc.compile()
    return nc


_NC_CACHE = {}


def _get_nc(route_a=N_ROUTE_A, pbase=PBASE, debug=False):
    key = (route_a, pbase, debug)
    if key not in _NC_CACHE:
        _NC_CACHE[key] = _build(route_a, pbase, debug)
    return _NC_CACHE[key]


def _prep_weights(w1, b1, w2, b2, w3, b3, w4, b4, w5, b5):
    s1 = np.sign(w1).astype(np.float32)  # [100,3,5,5]
    s2 = np.sign(w2).astype(np.float32)  # [16,100,5,5]
    s3 = np.sign(w3).astype(np.float32)  # [120,400]
    s4 = np.sign(w4).astype(np.float32)  # [84,120]
    s5 = np.sign(w5).astype(np.float32)  # [10,84]

    # conv1 lhsT rows: r = ci*25 + ky*5 + kx; cols padded 100 -> 112
    w1t = np.zeros((75, 112), np.float32)
    w1t[:, :100] = s1.transpose(1, 2, 3, 0).reshape(75, 100)
    w1l = w1t / LO_SCALE
    # conv2 lhsT: [ci, t=ky*5+kx, o] (plain signs; rhs is centered h1)
    w2t = np.ascontiguousarray(
        s2.transpose(1, 2, 3, 0).reshape(100, 25, 16)
    ).astype(np.float16)
    # fc1 taps: [c2, p, o] = s3[o, c2*25+p]
    w3t = np.ascontiguousarray(
        s3.reshape(120, 16, 25).transpose(1, 2, 0)
    ).astype(np.float16)
    w4t = np.ascontiguousarray(s4.T).astype(np.float16)
    w5t = np.ascontiguousarray(s5.T).astype(np.float16)

    def colvec(v, n):
        out = np.zeros((n, 1), np.float32)
        out[: len(v), 0] = v
        return out

    b1p = colvec(b1 + 1.0, 112)
    # b2 replicated across the 4 conv2 column-group partition strips
    b2p = np.zeros((112, 1), np.float32)
    for g in range(4):
        b2p[32 * g : 32 * g + 16, 0] = b2 + 1.0
    b3p = colvec(b3 + 1.0, 120)
    b4p = colvec(b4 + 1.0, 84)
    b5e = colvec(b5, 10)
    return {
        "w1t": w1t.astype(np.float16), "w1l": w1l.astype(FP8NP),
        "w2t": w2t, "w3t": w3t, "w4t": w4t, "w5t": w5t,
        "b1p": b1p, "b2p": b2p, "b3p": b3p, "b4p": b4p, "b5e": b5e,
    }


def kernel(x, w1, b1, w2, b2, w3, b3, w4, b4, w5, b5, _trace=False,
           _route_a=N_ROUTE_A, _pbase=PBASE, _debug=False):
    x = np.asarray(x, dtype=np.float32)
    wmap = _prep_weights(
        np.asarray(w1), np.asarray(b1), np.asarray(w2), np.asarray(b2),
        np.asarray(w3), np.asarray(b3), np.asarray(w4), np.asarray(b4),
        np.asarray(w5), np.asarray(b5),
    )
    nc = _get_nc(_route_a, _pbase, _debug)
    in_maps = []
    for c in range(NCORES):
        xs = x[c * BPC : (c + 1) * BPC]  # [256,3,32,32]
        xs = np.ascontiguousarray(
            xs.transpose(1, 0, 2, 3).reshape(3, BPC * 1024)
        )
        xh0 = np.zeros((3, XPAD), np.float16)
        xh0[:, : BPC * 1024] = xs.astype(np.float16)
        xl0 = np.zeros((3, XPAD), FP8NP)
        xl0[:, : BPC * 1024] = (
            (xs - xh0[:, : BPC * 1024].astype(np.float32)) * LO_SCALE
        ).astype(FP8NP)
        # pre-expand along ky (shift 32*ky) and trim each image's stream to
        # IMW=896 elements (rows y>=28 are never read by conv1)
        xh = np.zeros((15, XPADT), np.float16)
        xl = np.zeros((15, XPADT), FP8NP)
        for ci in range(3):
            for ky in range(5):
                s = 32 * ky
                a = ci * 5 + ky
                sh = xh0[ci, s : s + BPC * 1024].reshape(BPC, 1024)
                sl = xl0[ci, s : s + BPC * 1024].reshape(BPC, 1024)
                xh[a, : BPC * IMW] = sh[:, :IMW].ravel()
                xl[a, : BPC * IMW] = sl[:, :IMW].ravel()
        in_maps.append({"xh": xh, "xl": xl, **wmap})
    res = run_bass_kernel_spmd(
        nc, in_maps, list(range(NCORES)), trace=_trace
    )
    out = np.empty((NCORES * BPC, 10), np.float32)
    for c in range(NCORES):
        out[c * BPC : (c + 1) * BPC] = res.results[c]["y"].T
    if _trace:
        return out, res
    return out


# revision 41
# speedup vs baseline: 1.0202x; 1.0202x over previous
"""Trainium2 Bass kernel for nn_ExampleBinaryNet (binarized LeNet-style CNN).

Data parallel over 8 NeuronCores, 256 images each. Per core:
  conv1 (3->100, 5x5): im2col to K=75 rows (r = ky*15 + ci*5 + kx), built by
    ONE 4D-AP dma_start per 16-image tile (hi fp16 on the sync HWDGE ring,
    fp8 residual lo on the scalar ring) so the transfer spans all 75 dest
    partitions and engages ~11-16 SDMA engines. Two accumulating matmuls per
    half-image: fp16 hi + plain-fp8 lo (x = fp16(x) + 2^-6 fp8((x-hi)*64)).
  epilogue (hardtanh+maxpool folded): activations are stored centered,
    r = clip(z,-1,1) itself, via r = min(relu(z+1),2)-1; so conv weights
    stay plain signs and biases need no sign-sum corrections.
    Route D images: ACT evicts relu(P+b+1) with x-parity-deinterleaved
    write, then two fp16 tensor_tensor max ops (2x DVE mode) do the 2x2
    maxpool; route A images: DVE tensor_reduce(max) straight from PSUM then
    a tiny ACT. One batched tensor_scalar (min 2, sub 1) per tile finishes.
  conv2 (100->16, 5x5): 25 accumulating tap matmuls, K=100, 4-way PE column
    tiling into ONE shared single-bank PSUM tile; pool2 epilogue runs
    full-partition-span ops covering all 4 groups at once.
  fc1/fc2/fc3: fp16 matmuls (fc1 as 25 accumulating K=16 taps), N=256.
"""

import os
import sys

for _p in ("/opt/trn_rl_repo", "/root/.axon_site/_ro/trn_rl_repo"):
    if os.path.isdir(_p) and _p not in sys.path:
        sys.path.insert(0, _p)

import numpy as np
import ml_dtypes

import concourse.bass as bass
import concourse.tile as tile
from concourse import bacc, mybir
from concourse.bass_utils import run_bass_kernel_spmd

F32 = mybir.dt.float32
FP16 = mybir.dt.float16
FP8 = mybir.dt.float8e4
FP8NP = ml_dtypes.float8_e4m3

NCORES = 8
BPC = 256          # batch per core
NB = 16            # images per batch-tile
NT = BPC // NB     # batch-tiles per core
XPAD = BPC * 1024 + 1024   # flat padded per-channel image stream
# per-image trimmed stream: conv1 only reads offsets 0..895 of each 1024
# (y rows 28..31 are pool margin); host repacks images at 896 stride to cut
# im2col DMA bytes by 12.5%
IMW = 896
XPADT = BPC * IMW + 1024
N_ROUTE_A = 0      # images per tile pooled by DVE straight from PSUM
LO_SCALE = 64.0
# partition base for im2col/weight tiles. Would love 32 to spread the
# im2col DMA over all 16 SDMA engines, but LDWEIGHTS at base 32 may only
# span 32 partitions (BIR verifier), so K=75 weights must sit at base 0.
PBASE = 0


def _build(route_a=N_ROUTE_A, pbase=PBASE, debug=False):
    nc = bacc.Bacc("TRN2", target_bir_lowering=False, debug=False)

    # ---------------- DRAM I/O ----------------
    # xh/xl are host-pre-expanded along ky only: row a = ci*5 + ky holds the
    # channel stream shifted by 32*ky. The kx shifts come from a 3D DMA
    # access pattern [[XPAD,15],[1,5],[1,N]], giving the full 75-row im2col
    # (r = ci*25 + ky*5 + kx) per tile in one dma_start. 75 dest rows ->
    # 15 SDMA engines (5 rows each), while HBM reads stay inside a hot
    # 7.9 MB region (kx re-reads hit open rows).
    xh_d = nc.dram_tensor("xh", [15, XPADT], FP16, kind="ExternalInput")
    xl_d = nc.dram_tensor("xl", [15, XPADT], FP8, kind="ExternalInput")
    w1_d = nc.dram_tensor("w1t", [75, 112], FP16, kind="ExternalInput")
    w1l_d = nc.dram_tensor("w1l", [75, 112], FP8, kind="ExternalInput")
    w2_d = nc.dram_tensor("w2t", [100, 25, 16], FP16, kind="ExternalInput")
    w3_d = nc.dram_tensor("w3t", [16, 25, 120], FP16, kind="ExternalInput")
    w4_d = nc.dram_tensor("w4t", [120, 84], FP16, kind="ExternalInput")
    w5_d = nc.dram_tensor("w5t", [84, 10], FP16, kind="ExternalInput")
    b1p_d = nc.dram_tensor("b1p", [112, 1], F32, kind="ExternalInput")
    b2p_d = nc.dram_tensor("b2p", [112, 1], F32, kind="ExternalInput")
    b3p_d = nc.dram_tensor("b3p", [120, 1], F32, kind="ExternalInput")
    b4p_d = nc.dram_tensor("b4p", [84, 1], F32, kind="ExternalInput")
    b5e_d = nc.dram_tensor("b5e", [10, 1], F32, kind="ExternalInput")
    y_d = nc.dram_tensor("y", [10, BPC], F32, kind="ExternalOutput")
    if debug:
        dbg_r2 = nc.dram_tensor("dbg_r2", [100, NB, 196], FP16,
                                kind="ExternalOutput")
        dbg_r2p = nc.dram_tensor("dbg_r2p", [16, BPC, 25], FP16,
                                 kind="ExternalOutput")

    nA = route_a
    pb = pbase
    NJ = NB // 4  # images per conv2 column group
    # route is assigned per image-PAIR (route-D TT-maxes batch two images);
    # spread route-A pairs across the tile so DVE/ACT load interleaves
    npair = NB // 2
    na_pairs = nA // 2
    a_pairs = (
        {round(i * npair / na_pairs) for i in range(na_pairs)}
        if na_pairs > 0 else set()
    )

    with tile.TileContext(nc) as tc:
        with (
            tc.tile_pool(name="consts", bufs=1) as consts,
            tc.tile_pool(name="im_p", bufs=3) as im_p,
            tc.tile_pool(name="iml_p", bufs=2) as iml_p,
            tc.tile_pool(name="ep_p", bufs=2) as ep_p,
            tc.tile_pool(name="r2_p", bufs=2) as r2_p,
            tc.tile_pool(name="p2_p", bufs=2) as p2_p,
            tc.tile_pool(name="fc_p", bufs=1) as fc_p,
            tc.tile_pool(name="ps1_p", bufs=3, space="PSUM") as ps1_p,
            tc.tile_pool(name="ps2_p", bufs=1, space="PSUM") as ps2_p,
        ):
            # ---------------- constants ----------------
            w1full = consts.tile([pb + 75, 112], FP16, name="w1full")
            w1lfull = consts.tile([pb + 75, 112], FP8, name="w1lfull")
            w1sb = w1full[pb : pb + 75, :]
            w1lsb = w1lfull[pb : pb + 75, :]
            w2sb = consts.tile([100, 25, 16], FP16)
            w3sb = consts.tile([16, 25, 120], FP16)
            w4sb = consts.tile([120, 84], FP16)
            w5sb = consts.tile([84, 10], FP16)
            b1p = consts.tile([112, 1], F32)
            b2p = consts.tile([112, 1], F32)
            b3p = consts.tile([120, 1], F32)
            b4p = consts.tile([84, 1], F32)
            b5e = consts.tile([10, 1], F32)
            r2p = consts.tile([16, BPC, 25], FP16, name="r2p")
            # only conv1's weights/bias gate the first matmul; the rest load
            # behind the first im2col DMA to shorten the startup ramp
            for t_sb, t_d in [(w1sb, w1_d), (w1lsb, w1l_d), (b1p, b1p_d)]:
                nc.sync.dma_start(out=t_sb, in_=t_d[:])

            def load_late_consts():
                for t_sb, t_d in [
                    (w2sb, w2_d), (w3sb, w3_d), (w4sb, w4_d), (w5sb, w5_d),
                    (b2p, b2p_d), (b3p, b3p_d), (b4p, b4p_d), (b5e, b5e_d),
                ]:
                    nc.sync.dma_start(out=t_sb, in_=t_d[:])

            prev = None  # state for conv2 stage of previous batch-tile

            def conv2_block(pv):
                """conv2 + pool2 + fc-input epilogue for one batch-tile."""
                it, r2 = pv
                pg = ps2_p.tile([128, 512], F32, name=f"pg_{it}", tag="pg")
                # r2 viewed as [100, j, g, 14, 14] with local image b = 4j+g
                r2v = r2[:].rearrange("p (j g) (y x) -> p j g y x", g=4, x=14)
                for t in range(25):
                    ky, kx = divmod(t, 5)
                    for g in range(4):
                        rhs = r2v[:, :, g, ky : ky + 10, kx : kx + 10]
                        nc.tensor.matmul(
                            pg[32 * g : 32 * g + 16, 0 : 100 * NJ],
                            w2sb[:, t, :],
                            rhs,
                            start=(t == 0),
                            stop=(t == 24),
                            tile_position=(0, 32 * g),
                        )
                # one full-partition-span contiguous evict: relu(P2+(b2+1));
                # garbage partition strips (16..31 etc) are written but never
                # read downstream.
                ev2 = p2_p.tile([112, NJ, 10, 10], FP16, name=f"ev2_{it}",
                                tag="ev2")
                nc.scalar.activation(
                    out=ev2[:].rearrange("p j y x -> p (j y x)"),
                    in_=pg[0:112, 0 : 100 * NJ],
                    func=mybir.ActivationFunctionType.Relu,
                    bias=b2p[:],
                    scale=1.0,
                )
                # maxpool 2x2: x-pairs (strided, 1x) then y-pairs
                m1p = p2_p.tile([112, NJ, 10, 5], FP16, name=f"m1p_{it}",
                                tag="m1p")
                ev2v = ev2[:].rearrange("p j y (xa xb) -> p j y xa xb", xb=2)
                nc.vector.tensor_tensor(
                    m1p[:].rearrange("p j y xa -> p (j y xa)"),
                    ev2v[:, :, :, :, 0].rearrange("p j y xa -> p (j y xa)"),
                    ev2v[:, :, :, :, 1].rearrange("p j y xa -> p (j y xa)"),
                    mybir.AluOpType.max,
                )
                m2p = p2_p.tile([112, NJ, 5, 5], FP16, name=f"m2p_{it}",
                                tag="m2p")
                m1v = m1p[:].rearrange("p j (ya yb) xa -> p j ya yb xa", yb=2)
                nc.vector.tensor_tensor(
                    m2p[:], m1v[:, :, :, 0, :], m1v[:, :, :, 1, :],
                    mybir.AluOpType.max,
                )
                # r2p = min(m2, 2) - 1  (store h2 in [-1,1]); per-group
                # cross-partition remap into partitions 0..16
                r2pv = r2p[:].rearrange("p (t j g) f -> p t j g f", t=NT, g=4)
                for g in range(4):
                    nc.vector.tensor_scalar(
                        out=r2pv[:, it, :, g, :],
                        in0=m2p[32 * g : 32 * g + 16].rearrange(
                            "p j a b -> p j (a b)"
                        ),
                        scalar1=2.0,
                        scalar2=1.0,
                        op0=mybir.AluOpType.min,
                        op1=mybir.AluOpType.subtract,
                    )

            for it in range(NT):
                # -------- im2col: one rectangular dma_start each ----------
                base = it * NB * IMW
                im = im_p.tile([pb + 75, NB * IMW], FP16, name=f"im_{it}",
                               tag="im")
                iml = iml_p.tile([pb + 75, NB * IMW], FP8, name=f"iml_{it}",
                                 tag="iml")
                src_hi = bass.AP(
                    tensor=xh_d.ap().tensor,
                    offset=base,
                    ap=[[XPADT, 15], [1, 5], [1, NB * IMW]],
                )
                nc.sync.dma_start(out=im[pb : pb + 75, :], in_=src_hi)
                src_lo = bass.AP(
                    tensor=xl_d.ap().tensor,
                    offset=base,
                    ap=[[XPADT, 15], [1, 5], [1, NB * IMW]],
                )
                # same ring as hi: a 75-row DMA already spans ~15 of the 16
                # physical SDMA engines, so a second ring only time-shares
                # them at half rate (measured 13.7 vs 27 GB/s per slot)
                nc.sync.dma_start(out=iml[pb : pb + 75, :], in_=src_lo)
                if it == 0:
                    load_late_consts()

                # ---------------- conv2 of previous tile ----------------
                # emitted BEFORE this tile's conv1 so the 25 taps run dense
                # on the PE and the 4 column-tiled matmuls per tap actually
                # overlap (interleaving conv1 between them serializes all)
                if prev is not None:
                    conv2_block(prev)

                # ---------------- conv1 + pool1 epilogue ----------------
                r2u = ep_p.tile([100, NB, 196], FP16, name=f"r2u_{it}",
                                tag="r2u")
                r2 = r2_p.tile([100, NB, 196], FP16, name=f"r2_{it}", tag="r2")

                for pi in range(npair):
                    # pair the hi (then lo) matmuls of two images so the PE
                    # keeps the same stationary weights across 4 consecutive
                    # matmuls instead of swapping hi/lo every matmul
                    pair = (2 * pi, 2 * pi + 1)
                    ps1s = {}
                    for bb in pair:
                        ps1s[bb] = ps1_p.tile(
                            [112, 2, 512], F32, name=f"ps1_{it}_{bb}",
                            tag="ps1"
                        )
                    for w_sb, i_t, first in ((w1sb, im, True),
                                             (w1lsb, iml, False)):
                        for bb in pair:
                            ib = i_t[
                                pb : pb + 75,
                                bb * IMW : bb * IMW + IMW,
                            ].rearrange("p (y w) -> p y w", w=32)
                            for h in range(2):
                                nc.tensor.matmul(
                                    ps1s[bb][:, h, 0:392],
                                    w_sb,
                                    ib[:, 14 * h : 14 * h + 14, 0:28],
                                    start=first,
                                    stop=not first,
                                )
                    if pi in a_pairs:
                        # route A: DVE maxpool straight from PSUM (per bank),
                        # then ACT relu(. + b1 + 1)
                        for b in pair:
                            praw = ep_p.tile([100, 2, 7, 14], FP16,
                                             name=f"praw_{it}_{b}",
                                             tag="praw")
                            for h in range(2):
                                nc.vector.tensor_reduce(
                                    out=praw[:, h],
                                    in_=ps1s[b][0:100, h, 0:392].rearrange(
                                        "p (y a x b) -> p y x a b",
                                        y=7, a=2, b=2
                                    ),
                                    axis=mybir.AxisListType.XY,
                                    op=mybir.AluOpType.max,
                                )
                            nc.scalar.activation(
                                out=r2u[:, b, :],
                                in_=praw[:].rearrange("p h y x -> p (h y x)"),
                                func=mybir.ActivationFunctionType.Relu,
                                bias=b1p[0:100],
                                scale=1.0,
                            )
                    else:
                        # route D: per-image ACT evict relu(P + b1 + 1) with
                        # x-parity deinterleave via a write AP whose inner
                        # run is 14 contiguous elements; then ONE 2x-mode
                        # TT-max per pool stage covering both images
                        ev1 = ep_p.tile([100, 2, 2, 28, 14], FP16,
                                        name=f"ev1_{it}_{pi}", tag="ev1")
                        for i, b in enumerate(pair):
                            out_ap = ev1[:, i].rearrange(
                                "p xb (h y) xa -> p xb h y xa", h=2
                            )
                            in_ap = ps1s[b][0:100, :, 0:392].rearrange(
                                "p h (y xa xb) -> p xb h y xa", y=14, xb=2
                            )
                            nc.scalar.activation(
                                out=out_ap, in_=in_ap,
                                func=mybir.ActivationFunctionType.Relu,
                                bias=b1p[0:100],
                                scale=1.0,
                            )
                        m1 = ep_p.tile([100, 2, 28, 14], FP16,
                                       name=f"m1_{it}_{pi}", tag="m1")
                        nc.vector.tensor_tensor(
                            m1[:],
                            ev1[:, :, 0],
                            ev1[:, :, 1],
                            mybir.AluOpType.max,
                        )
                        m1v = m1[:].rearrange(
                            "p i (ya yb) x -> p i ya yb x", yb=2
                        )
                        nc.vector.tensor_tensor(
                            r2u[:, 2 * pi : 2 * pi + 2, :].rearrange(
                                "p i (y x) -> p i y x", x=14
                            ),
                            m1v[:, :, :, 0, :],
                            m1v[:, :, :, 1, :],
                            mybir.AluOpType.max,
                        )

                # batched: r2 = min(r2u, 2) - 1  (store h1 in [-1,1])
                nc.vector.tensor_scalar(
                    out=r2[:].rearrange("p b f -> p (b f)"),
                    in0=r2u[:].rearrange("p b f -> p (b f)"),
                    scalar1=2.0,
                    scalar2=1.0,
                    op0=mybir.AluOpType.min,
                    op1=mybir.AluOpType.subtract,
                )

                if debug and it == 0:
                    nc.sync.dma_start(out=dbg_r2[:], in_=r2[:])

                prev = (it, r2)

            conv2_block(prev)

            # ---------------- fully connected layers ----------------
            if debug:
                nc.sync.dma_start(out=dbg_r2p[:], in_=r2p[:])
            ps3 = ps1_p.tile([120, BPC], F32, name="ps3", tag="ps1")
            for p in range(25):
                nc.tensor.matmul(
                    ps3[:],
                    w3sb[:, p, :],
                    r2p[:, :, p],
                    start=(p == 0),
                    stop=(p == 24),
                )
            u3 = fc_p.tile([120, BPC], F32)
            nc.scalar.activation(
                out=u3[:], in_=ps3[:],
                func=mybir.ActivationFunctionType.Relu,
                bias=b3p[:], scale=1.0,
            )
            r3 = fc_p.tile([120, BPC], FP16)
            nc.vector.tensor_scalar(
                out=r3[:], in0=u3[:], scalar1=2.0, scalar2=1.0,
                op0=mybir.AluOpType.min, op1=mybir.AluOpType.subtract,
            )

            ps4 = ps1_p.tile([84, BPC], F32, name="ps4", tag="ps1")
            nc.tensor.matmul(ps4[:], w4sb[:], r3[:], start=True, stop=True)
            u4 = fc_p.tile([84, BPC], F32)
            nc.scalar.activation(
                out=u4[:], in_=ps4[:],
                func=mybir.ActivationFunctionType.Relu,
                bias=b4p[:], scale=1.0,
            )
            r4 = fc_p.tile([84, BPC], FP16)
            nc.vector.tensor_scalar(
                out=r4[:], in0=u4[:], scalar1=2.0, scalar2=1.0,
                op0=mybir.AluOpType.min, op1=mybir.AluOpType.subtract,
            )

            ps5 = ps1_p.tile([10, BPC], F32, name="ps5", tag="ps1")
            nc.tensor.matmul(ps5[:], w5sb[:], r4[:], start=True, stop=True)
            y_sb = fc_p.tile([10, BPC], F32)
            nc.vector.tensor_scalar_add(y_sb[:], ps5[:], b5e[:])
            nc.sync.dma_start(out=y_d[:], in_=y_sb[:])

    nc.compile()
    return nc


_NC_CACHE = {}


def _get_nc(route_a=N_ROUTE_A, pbase=PBASE, debug=False):
    key = (route_a, pbase, debug)
    if key not in _NC_CACHE:
        _NC_CACHE[key] = _build(route_a, pbase, debug)
    return _NC_CACHE[key]


def _prep_weights(w1, b1, w2, b2, w3, b3, w4, b4, w5, b5):
    s1 = np.sign(w1).astype(np.float32)  # [100,3,5,5]
    s2 = np.sign(w2).astype(np.float32)  # [16,100,5,5]
    s3 = np.sign(w3).astype(np.float32)  # [120,400]
    s4 = np.sign(w4).astype(np.float32)  # [84,120]
    s5 = np.sign(w5).astype(np.float32)  # [10,84]

    # conv1 lhsT rows: r = ci*25 + ky*5 + kx; cols padded 100 -> 112
    w1t = np.zeros((75, 112), np.float32)
    w1t[:, :100] = s1.transpose(1, 2, 3, 0).reshape(75, 100)
    w1l = w1t / LO_SCALE
    # conv2 lhsT: [ci, t=ky*5+kx, o] (plain signs; rhs is centered h1)
    w2t = np.ascontiguousarray(
        s2.transpose(1, 2, 3, 0).reshape(100, 25, 16)
    ).astype(np.float16)
    # fc1 taps: [c2, p, o] = s3[o, c2*25+p]
    w3t = np.ascontiguousarray(
        s3.reshape(120, 16, 25).transpose(1, 2, 0)
    ).astype(np.float16)
    w4t = np.ascontiguousarray(s4.T).astype(np.float16)
    w5t = np.ascontiguousarray(s5.T).astype(np.float16)

    def colvec(v, n):
        out = np.zeros((n, 1), np.float32)
        out[: len(v), 0] = v
        return out

    b1p = colvec(b1 + 1.0, 112)
    # b2 replicated across the 4 conv2 column-group partition strips
    b2p = np.zeros((112, 1), np.float32)
    for g in range(4):
        b2p[32 * g : 32 * g + 16, 0] = b2 + 1.0
    b3p = colvec(b3 + 1.0, 120)
    b4p = colvec(b4 + 1.0, 84)
    b5e = colvec(b5, 10)
    return {
        "w1t": w1t.astype(np.float16), "w1l": w1l.astype(FP8NP),
        "w2t": w2t, "w3t": w3t, "w4t": w4t, "w5t": w5t,
        "b1p": b1p, "b2p": b2p, "b3p": b3p, "b4p": b4p, "b5e": b5e,
    }


def kernel(x, w1, b1, w2, b2, w3, b3, w4, b4, w5, b5, _trace=False,
           _route_a=N_ROUTE_A, _pbase=PBASE, _debug=False):
    x = np.asarray(x, dtype=np.float32)
    wmap = _prep_weights(
        np.asarray(w1), np.asarray(b1), np.asarray(w2), np.asarray(b2),
        np.asarray(w3), np.asarray(b3), np.asarray(w4), np.asarray(b4),
        np.asarray(w5), np.asarray(b5),
    )
    nc = _get_nc(_route_a, _pbase, _debug)
    in_maps = []
    for c in range(NCORES):
        xs = x[c * BPC : (c + 1) * BPC]  # [256,3,32,32]
        xs = np.ascontiguousarray(
            xs.transpose(1, 0, 2, 3).reshape(3, BPC * 1024)
        )
        xh0 = np.zeros((3, XPAD), np.float16)
        xh0[:, : BPC * 1024] = xs.astype(np.float16)
        xl0 = np.zeros((3, XPAD), FP8NP)
        xl0[:, : BPC * 1024] = (
            (xs - xh0[:, : BPC * 1024].astype(np.float32)) * LO_SCALE
        ).astype(FP8NP)
        # pre-expand along ky (shift 32*ky) and trim each image's stream to
        # IMW=896 elements (rows y>=28 are never read by conv1)
        xh = np.zeros((15, XPADT), np.float16)
        xl = np.zeros((15, XPADT), FP8NP)
        for ci in range(3):
            for ky in range(5):
                s = 32 * ky
                a = ci * 5 + ky
                sh = xh0[ci, s : s + BPC * 1024].reshape(BPC, 1024)
                sl = xl0[ci, s : s + BPC * 1024].reshape(BPC, 1024)
                xh[a, : BPC * IMW] = sh[:, :IMW].ravel()
                xl[a, : BPC * IMW] = sl[:, :IMW].ravel()
        in_maps.append({"xh": xh, "xl": xl, **wmap})
    res = run_bass_kernel_spmd(
        nc, in_maps, list(range(NCORES)), trace=_trace
    )
    out = np.empty((NCORES * BPC, 10), np.float32)
    for c in range(NCORES):
        out[c * BPC : (c + 1) * BPC] = res.results[c]["y"].T
    if _trace:
        return out, res
    return out


# revision 43
# speedup vs baseline: 1.0733x; 1.0521x over previous
"""Trainium2 Bass kernel for nn_ExampleBinaryNet (binarized LeNet-style CNN).

Data parallel over 8 NeuronCores, 256 images each. Per core:
  conv1 (3->100, 5x5): im2col to K=75 rows (r = ky*15 + ci*5 + kx), built by
    ONE 4D-AP dma_start per 16-image tile (hi fp16 on the sync HWDGE ring,
    fp8 residual lo on the scalar ring) so the transfer spans all 75 dest
    partitions and engages ~11-16 SDMA engines. Two accumulating matmuls per
    half-image: fp16 hi + plain-fp8 lo (x = fp16(x) + 2^-6 fp8((x-hi)*64)).
  epilogue (hardtanh+maxpool folded): activations are stored centered,
    r = clip(z,-1,1) itself, via r = min(relu(z+1),2)-1; so conv weights
    stay plain signs and biases need no sign-sum corrections.
    Route D images: ACT evicts relu(P+b+1) with x-parity-deinterleaved
    write, then two fp16 tensor_tensor max ops (2x DVE mode) do the 2x2
    maxpool; route A images: DVE tensor_reduce(max) straight from PSUM then
    a tiny ACT. One batched tensor_scalar (min 2, sub 1) per tile finishes.
  conv2 (100->16, 5x5): 25 accumulating tap matmuls, K=100, 4-way PE column
    tiling into ONE shared single-bank PSUM tile; pool2 epilogue runs
    full-partition-span ops covering all 4 groups at once.
  fc1/fc2/fc3: fp16 matmuls (fc1 as 25 accumulating K=16 taps), N=256.
"""

import os
import sys

for _p in ("/opt/trn_rl_repo", "/root/.axon_site/_ro/trn_rl_repo"):
    if os.path.isdir(_p) and _p not in sys.path:
        sys.path.insert(0, _p)

import numpy as np
import ml_dtypes

import concourse.bass as bass
import concourse.tile as tile
from concourse import bacc, mybir
from concourse.bass_utils import run_bass_kernel_spmd

F32 = mybir.dt.float32
FP16 = mybir.dt.float16
FP8 = mybir.dt.float8e4
FP8NP = ml_dtypes.float8_e4m3

NCORES = 8
BPC = 256          # batch per core
NB = 16            # images per batch-tile
NT = BPC // NB     # batch-tiles per core
XPAD = BPC * 1024 + 1024   # flat padded per-channel image stream
# per-image trimmed stream: conv1 only reads offsets 0..895 of each 1024
# (y rows 28..31 are pool margin); host repacks images at 896 stride to cut
# im2col DMA bytes by 12.5%
IMW = 896
XPADT = BPC * IMW + 1024
N_ROUTE_A = 0      # images per tile pooled by DVE straight from PSUM
LO_SCALE = 64.0
# partition base for im2col/weight tiles. Would love 32 to spread the
# im2col DMA over all 16 SDMA engines, but LDWEIGHTS at base 32 may only
# span 32 partitions (BIR verifier), so K=75 weights must sit at base 0.
PBASE = 0


def _build(route_a=N_ROUTE_A, pbase=PBASE, debug=False):
    nc = bacc.Bacc("TRN2", target_bir_lowering=False, debug=False)

    # ---------------- DRAM I/O ----------------
    # xh/xl are host-pre-expanded along ky only: row a = ci*5 + ky holds the
    # channel stream shifted by 32*ky. The kx shifts come from a 3D DMA
    # access pattern [[XPAD,15],[1,5],[1,N]], giving the full 75-row im2col
    # (r = ci*25 + ky*5 + kx) per tile in one dma_start. 75 dest rows ->
    # 15 SDMA engines (5 rows each), while HBM reads stay inside a hot
    # 7.9 MB region (kx re-reads hit open rows).
    xh_d = nc.dram_tensor("xh", [15, XPADT], FP16, kind="ExternalInput")
    xl_d = nc.dram_tensor("xl", [15, XPADT], FP8, kind="ExternalInput")
    w1_d = nc.dram_tensor("w1t", [75, 112], FP16, kind="ExternalInput")
    w1l_d = nc.dram_tensor("w1l", [75, 112], FP8, kind="ExternalInput")
    w2_d = nc.dram_tensor("w2t", [100, 25, 16], FP16, kind="ExternalInput")
    w3_d = nc.dram_tensor("w3t", [16, 25, 120], FP16, kind="ExternalInput")
    w4_d = nc.dram_tensor("w4t", [120, 84], FP16, kind="ExternalInput")
    w5_d = nc.dram_tensor("w5t", [84, 10], FP16, kind="ExternalInput")
    b1p_d = nc.dram_tensor("b1p", [112, 1], F32, kind="ExternalInput")
    b2p_d = nc.dram_tensor("b2p", [112, 1], F32, kind="ExternalInput")
    b3p_d = nc.dram_tensor("b3p", [120, 1], F32, kind="ExternalInput")
    b4p_d = nc.dram_tensor("b4p", [84, 1], F32, kind="ExternalInput")
    b5e_d = nc.dram_tensor("b5e", [10, 1], F32, kind="ExternalInput")
    y_d = nc.dram_tensor("y", [10, BPC], F32, kind="ExternalOutput")
    if debug:
        dbg_r2 = nc.dram_tensor("dbg_r2", [100, NB, 196], FP16,
                                kind="ExternalOutput")
        dbg_r2p = nc.dram_tensor("dbg_r2p", [16, BPC, 25], FP16,
                                 kind="ExternalOutput")

    nA = route_a
    pb = pbase
    NJ = NB // 4  # images per conv2 column group
    # route is assigned per image-PAIR (route-D TT-maxes batch two images);
    # spread route-A pairs across the tile so DVE/ACT load interleaves
    npair = NB // 2
    na_pairs = nA // 2
    a_pairs = (
        {round(i * npair / na_pairs) for i in range(na_pairs)}
        if na_pairs > 0 else set()
    )

    with tile.TileContext(nc) as tc:
        with (
            tc.tile_pool(name="consts", bufs=1) as consts,
            tc.tile_pool(name="im_p", bufs=3) as im_p,
            tc.tile_pool(name="iml_p", bufs=2) as iml_p,
            tc.tile_pool(name="ep_p", bufs=2) as ep_p,
            tc.tile_pool(name="r2_p", bufs=2) as r2_p,
            tc.tile_pool(name="p2_p", bufs=2) as p2_p,
            tc.tile_pool(name="fc_p", bufs=1) as fc_p,
            tc.tile_pool(name="ps1_p", bufs=3, space="PSUM") as ps1_p,
            tc.tile_pool(name="ps2_p", bufs=1, space="PSUM") as ps2_p,
        ):
            # ---------------- constants ----------------
            w1full = consts.tile([pb + 75, 112], FP16, name="w1full")
            w1lfull = consts.tile([pb + 75, 112], FP8, name="w1lfull")
            w1sb = w1full[pb : pb + 75, :]
            w1lsb = w1lfull[pb : pb + 75, :]
            w2sb = consts.tile([100, 25, 16], FP16)
            w3sb = consts.tile([16, 25, 120], FP16)
            w4sb = consts.tile([120, 84], FP16)
            w5sb = consts.tile([84, 10], FP16)
            b1p = consts.tile([112, 1], F32)
            b2p = consts.tile([112, 1], F32)
            b3p = consts.tile([120, 1], F32)
            b4p = consts.tile([84, 1], F32)
            b5e = consts.tile([10, 1], F32)
            r2p = consts.tile([16, BPC, 25], FP16, name="r2p")
            # only conv1's weights/bias gate the first matmul; the rest load
            # behind the first im2col DMA to shorten the startup ramp
            for t_sb, t_d in [(w1sb, w1_d), (w1lsb, w1l_d), (b1p, b1p_d)]:
                nc.sync.dma_start(out=t_sb, in_=t_d[:])

            def load_late_consts():
                for t_sb, t_d in [
                    (w2sb, w2_d), (w3sb, w3_d), (w4sb, w4_d), (w5sb, w5_d),
                    (b2p, b2p_d), (b3p, b3p_d), (b4p, b4p_d), (b5e, b5e_d),
                ]:
                    nc.sync.dma_start(out=t_sb, in_=t_d[:])

            prev = None  # state for conv2 stage of previous batch-tile

            def conv2_block(pv):
                """conv2 + pool2 + fc-input epilogue for one batch-tile."""
                it, r2 = pv
                pg = ps2_p.tile([128, 512], F32, name=f"pg_{it}", tag="pg")
                # r2 viewed as [100, j, g, 14, 14] with local image b = 4j+g
                r2v = r2[:].rearrange("p (j g) (y x) -> p j g y x", g=4, x=14)
                for t in range(25):
                    ky, kx = divmod(t, 5)
                    for g in range(4):
                        rhs = r2v[:, :, g, ky : ky + 10, kx : kx + 10]
                        nc.tensor.matmul(
                            pg[32 * g : 32 * g + 16, 0 : 100 * NJ],
                            w2sb[:, t, :],
                            rhs,
                            start=(t == 0),
                            stop=(t == 24),
                            tile_position=(0, 32 * g),
                        )
                # one full-partition-span contiguous evict: relu(P2+(b2+1));
                # garbage partition strips (16..31 etc) are written but never
                # read downstream.
                ev2 = p2_p.tile([112, NJ, 10, 10], FP16, name=f"ev2_{it}",
                                tag="ev2")
                nc.scalar.activation(
                    out=ev2[:].rearrange("p j y x -> p (j y x)"),
                    in_=pg[0:112, 0 : 100 * NJ],
                    func=mybir.ActivationFunctionType.Relu,
                    bias=b2p[:],
                    scale=1.0,
                )
                # maxpool 2x2: x-pairs (strided, 1x) then y-pairs
                m1p = p2_p.tile([112, NJ, 10, 5], FP16, name=f"m1p_{it}",
                                tag="m1p")
                ev2v = ev2[:].rearrange("p j y (xa xb) -> p j y xa xb", xb=2)
                nc.vector.tensor_tensor(
                    m1p[:].rearrange("p j y xa -> p (j y xa)"),
                    ev2v[:, :, :, :, 0].rearrange("p j y xa -> p (j y xa)"),
                    ev2v[:, :, :, :, 1].rearrange("p j y xa -> p (j y xa)"),
                    mybir.AluOpType.max,
                )
                m2p = p2_p.tile([112, NJ, 5, 5], FP16, name=f"m2p_{it}",
                                tag="m2p")
                m1v = m1p[:].rearrange("p j (ya yb) xa -> p j ya yb xa", yb=2)
                nc.vector.tensor_tensor(
                    m2p[:], m1v[:, :, :, 0, :], m1v[:, :, :, 1, :],
                    mybir.AluOpType.max,
                )
                # r2p = min(m2, 2) - 1  (store h2 in [-1,1]); per-group
                # cross-partition remap into partitions 0..16
                r2pv = r2p[:].rearrange("p (t j g) f -> p t j g f", t=NT, g=4)
                for g in range(4):
                    nc.vector.tensor_scalar(
                        out=r2pv[:, it, :, g, :],
                        in0=m2p[32 * g : 32 * g + 16].rearrange(
                            "p j a b -> p j (a b)"
                        ),
                        scalar1=2.0,
                        scalar2=1.0,
                        op0=mybir.AluOpType.min,
                        op1=mybir.AluOpType.subtract,
                    )

            for it in range(NT):
                # -------- im2col: one rectangular dma_start each ----------
                # two half-tiles per batch-tile so conv1 of images 0-7 can
                # start while images 8-15 still stream in. Single ring: a
                # 75-row DMA already spans ~15 of the 16 physical SDMA
                # engines; a second ring only time-shares them at half rate
                # (measured 13.7 vs 27 GB/s per slot).
                HB = NB // 2
                ims, imls = [], []
                for hh in range(2):
                    base = (it * NB + hh * HB) * IMW
                    imh = im_p.tile([pb + 75, HB * IMW], FP16,
                                    name=f"im_{it}_{hh}", tag=f"im{hh}")
                    imlh = iml_p.tile([pb + 75, HB * IMW], FP8,
                                      name=f"iml_{it}_{hh}", tag=f"iml{hh}")
                    src_hi = bass.AP(
                        tensor=xh_d.ap().tensor,
                        offset=base,
                        ap=[[XPADT, 15], [1, 5], [1, HB * IMW]],
                    )
                    nc.sync.dma_start(out=imh[pb : pb + 75, :], in_=src_hi)
                    src_lo = bass.AP(
                        tensor=xl_d.ap().tensor,
                        offset=base,
                        ap=[[XPADT, 15], [1, 5], [1, HB * IMW]],
                    )
                    nc.sync.dma_start(out=imlh[pb : pb + 75, :], in_=src_lo)
                    ims.append(imh)
                    imls.append(imlh)
                    if it == 0 and hh == 0:
                        load_late_consts()

                # ---------------- conv2 of previous tile ----------------
                # emitted BEFORE this tile's conv1 so the 25 taps run dense
                # on the PE and the 4 column-tiled matmuls per tap actually
                # overlap (interleaving conv1 between them serializes all)
                if prev is not None:
                    conv2_block(prev)

                # ---------------- conv1 + pool1 epilogue ----------------
                r2u = ep_p.tile([100, NB, 196], FP16, name=f"r2u_{it}",
                                tag="r2u")
                r2 = r2_p.tile([100, NB, 196], FP16, name=f"r2_{it}", tag="r2")

                for pi in range(npair):
                    # pair the hi (then lo) matmuls of two images so the PE
                    # keeps the same stationary weights across 4 consecutive
                    # matmuls instead of swapping hi/lo every matmul
                    pair = (2 * pi, 2 * pi + 1)
                    ps1s = {}
                    for bb in pair:
                        ps1s[bb] = ps1_p.tile(
                            [112, 2, 512], F32, name=f"ps1_{it}_{bb}",
                            tag="ps1"
                        )
                    hh = (2 * pi) // (NB // 2)
                    for w_sb, i_t, first in ((w1sb, ims[hh], True),
                                             (w1lsb, imls[hh], False)):
                        for bb in pair:
                            lb = bb - hh * (NB // 2)
                            ib = i_t[
                                pb : pb + 75,
                                lb * IMW : lb * IMW + IMW,
                            ].rearrange("p (y w) -> p y w", w=32)
                            for h in range(2):
                                nc.tensor.matmul(
                                    ps1s[bb][:, h, 0:392],
                                    w_sb,
                                    ib[:, 14 * h : 14 * h + 14, 0:28],
                                    start=first,
                                    stop=not first,
                                )
                    if pi in a_pairs:
                        # route A: DVE maxpool straight from PSUM (per bank),
                        # then ACT relu(. + b1 + 1)
                        for b in pair:
                            praw = ep_p.tile([100, 2, 7, 14], FP16,
                                             name=f"praw_{it}_{b}",
                                             tag="praw")
                            for h in range(2):
                                nc.vector.tensor_reduce(
                                    out=praw[:, h],
                                    in_=ps1s[b][0:100, h, 0:392].rearrange(
                                        "p (y a x b) -> p y x a b",
                                        y=7, a=2, b=2
                                    ),
                                    axis=mybir.AxisListType.XY,
                                    op=mybir.AluOpType.max,
                                )
                            nc.scalar.activation(
                                out=r2u[:, b, :],
                                in_=praw[:].rearrange("p h y x -> p (h y x)"),
                                func=mybir.ActivationFunctionType.Relu,
                                bias=b1p[0:100],
                                scale=1.0,
                            )
                    else:
                        # route D: per-image ACT evict relu(P + b1 + 1) with
                        # x-parity deinterleave via a write AP whose inner
                        # run is 14 contiguous elements; then ONE 2x-mode
                        # TT-max per pool stage covering both images
                        ev1 = ep_p.tile([100, 2, 2, 28, 14], FP16,
                                        name=f"ev1_{it}_{pi}", tag="ev1")
                        for i, b in enumerate(pair):
                            out_ap = ev1[:, i].rearrange(
                                "p xb (h y) xa -> p xb h y xa", h=2
                            )
                            in_ap = ps1s[b][0:100, :, 0:392].rearrange(
                                "p h (y xa xb) -> p xb h y xa", y=14, xb=2
                            )
                            nc.scalar.activation(
                                out=out_ap, in_=in_ap,
                                func=mybir.ActivationFunctionType.Relu,
                                bias=b1p[0:100],
                                scale=1.0,
                            )
                        m1 = ep_p.tile([100, 2, 28, 14], FP16,
                                       name=f"m1_{it}_{pi}", tag="m1")
                        nc.vector.tensor_tensor(
                            m1[:],
                            ev1[:, :, 0],
                            ev1[:, :, 1],
                            mybir.AluOpType.max,
                        )
                        m1v = m1[:].rearrange(
                            "p i (ya yb) x -> p i ya yb x", yb=2
                        )
                        nc.vector.tensor_tensor(
                            r2u[:, 2 * pi : 2 * pi + 2, :].rearrange(
                                "p i (y x) -> p i y x", x=14
                            ),
                            m1v[:, :, :, 0, :],
                            m1v[:, :, :, 1, :],
                            mybir.AluOpType.max,
                        )

                # batched: r2 = min(r2u, 2) - 1  (store h1 in [-1,1])
                nc.vector.tensor_scalar(
                    out=r2[:].rearrange("p b f -> p (b f)"),
                    in0=r2u[:].rearrange("p b f -> p (b f)"),
                    scalar1=2.0,
                    scalar2=1.0,
                    op0=mybir.AluOpType.min,
                    op1=mybir.AluOpType.subtract,
                )

                if debug and it == 0:
                    nc.sync.dma_start(out=dbg_r2[:], in_=r2[:])

                prev = (it, r2)

            conv2_block(prev)

            # ---------------- fully connected layers ----------------
            if debug:
                nc.sync.dma_start(out=dbg_r2p[:], in_=r2p[:])
            ps3 = ps1_p.tile([120, BPC], F32, name="ps3", tag="ps1")
            for p in range(25):
                nc.tensor.matmul(
                    ps3[:],
                    w3sb[:, p, :],
                    r2p[:, :, p],
                    start=(p == 0),
                    stop=(p == 24),
                )
            u3 = fc_p.tile([120, BPC], F32)
            nc.scalar.activation(
                out=u3[:], in_=ps3[:],
                func=mybir.ActivationFunctionType.Relu,
                bias=b3p[:], scale=1.0,
            )
            r3 = fc_p.tile([120, BPC], FP16)
            nc.vector.tensor_scalar(
                out=r3[:], in0=u3[:], scalar1=2.0, scalar2=1.0,
                op0=mybir.AluOpType.min, op1=mybir.AluOpType.subtract,
            )

            ps4 = ps1_p.tile([84, BPC], F32, name="ps4", tag="ps1")
            nc.tensor.matmul(ps4[:], w4sb[:], r3[:], start=True, stop=True)
            u4 = fc_p.tile([84, BPC], F32)
            nc.scalar.activation(
                out=u4[:], in_=ps4[:],
                func=mybir.ActivationFunctionType.Relu,
                bias=b4p[:], scale=1.0,
            )
            r4 = fc_p.tile([84, BPC], FP16)
            nc.vector.tensor_scalar(
                out=r4[:], in0=u4[:], scalar1=2.0, scalar2=1.0,
                op0=mybir.AluOpType.min, op1=mybir.AluOpType.subtract,
            )

            ps5 = ps1_p.tile([10, BPC], F32, name="ps5", tag="ps1")
            nc.tensor.matmul(ps5[:], w5sb[:], r4[:], start=True, stop=True)
            y_sb = fc_p.tile([10, BPC], F32)
            nc.vector.tensor_scalar_add(y_sb[:], ps5[:], b5e[:])
            nc.sync.dma_start(out=y_d[:], in_=y_sb[:])

    nc.compile()
    return nc


_NC_CACHE = {}


def _get_nc(route_a=N_ROUTE_A, pbase=PBASE, debug=False):
    key = (route_a, pbase, debug)
    if key not in _NC_CACHE:
        _NC_CACHE[key] = _build(route_a, pbase, debug)
    return _NC_CACHE[key]


def _prep_weights(w1, b1, w2, b2, w3, b3, w4, b4, w5, b5):
    s1 = np.sign(w1).astype(np.float32)  # [100,3,5,5]
    s2 = np.sign(w2).astype(np.float32)  # [16,100,5,5]
    s3 = np.sign(w3).astype(np.float32)  # [120,400]
    s4 = np.sign(w4).astype(np.float32)  # [84,120]
    s5 = np.sign(w5).astype(np.float32)  # [10,84]

    # conv1 lhsT rows: r = ci*25 + ky*5 + kx; cols padded 100 -> 112
    w1t = np.zeros((75, 112), np.float32)
    w1t[:, :100] = s1.transpose(1, 2, 3, 0).reshape(75, 100)
    w1l = w1t / LO_SCALE
    # conv2 lhsT: [ci, t=ky*5+kx, o] (plain signs; rhs is centered h1)
    w2t = np.ascontiguousarray(
        s2.transpose(1, 2, 3, 0).reshape(100, 25, 16)
    ).astype(np.float16)
    # fc1 taps: [c2, p, o] = s3[o, c2*25+p]
    w3t = np.ascontiguousarray(
        s3.reshape(120, 16, 25).transpose(1, 2, 0)
    ).astype(np.float16)
    w4t = np.ascontiguousarray(s4.T).astype(np.float16)
    w5t = np.ascontiguousarray(s5.T).astype(np.float16)

    def colvec(v, n):
        out = np.zeros((n, 1), np.float32)
        out[: len(v), 0] = v
        return out

    b1p = colvec(b1 + 1.0, 112)
    # b2 replicated across the 4 conv2 column-group partition strips
    b2p = np.zeros((112, 1), np.float32)
    for g in range(4):
        b2p[32 * g : 32 * g + 16, 0] = b2 + 1.0
    b3p = colvec(b3 + 1.0, 120)
    b4p = colvec(b4 + 1.0, 84)
    b5e = colvec(b5, 10)
    return {
        "w1t": w1t.astype(np.float16), "w1l": w1l.astype(FP8NP),
        "w2t": w2t, "w3t": w3t, "w4t": w4t, "w5t": w5t,
        "b1p": b1p, "b2p": b2p, "b3p": b3p, "b4p": b4p, "b5e": b5e,
    }


def kernel(x, w1, b1, w2, b2, w3, b3, w4, b4, w5, b5, _trace=False,
           _route_a=N_ROUTE_A, _pbase=PBASE, _debug=False):
    x = np.asarray(x, dtype=np.float32)
    wmap = _prep_weights(
        np.asarray(w1), np.asarray(b1), np.asarray(w2), np.asarray(b2),
        np.asarray(w3), np.asarray(b3), np.asarray(w4), np.asarray(b4),
        np.asarray(w5), np.asarray(b5),
    )
    nc = _get_nc(_route_a, _pbase, _debug)
    in_maps = []
    for c in range(NCORES):
        xs = x[c * BPC : (c + 1) * BPC]  # [256,3,32,32]
        xs = np.ascontiguousarray(
            xs.transpose(1, 0, 2, 3).reshape(3, BPC * 1024)
        )
        xh0 = np.zeros((3, XPAD), np.float16)
        xh0[:, : BPC * 1024] = xs.astype(np.float16)
        xl0 = np.zeros((3, XPAD), FP8NP)
        xl0[:, : BPC * 1024] = (
            (xs - xh0[:, : BPC * 1024].astype(np.float32)) * LO_SCALE
        ).astype(FP8NP)
        # pre-expand along ky (shift 32*ky) and trim each image's stream to
        # IMW=896 elements (rows y>=28 are never read by conv1)
        xh = np.zeros((15, XPADT), np.float16)
        xl = np.zeros((15, XPADT), FP8NP)
        for ci in range(3):
            for ky in range(5):
                s = 32 * ky
                a = ci * 5 + ky
                sh = xh0[ci, s : s + BPC * 1024].reshape(BPC, 1024)
                sl = xl0[ci, s : s + BPC * 1024].reshape(BPC, 1024)
                xh[a, : BPC * IMW] = sh[:, :IMW].ravel()
                xl[a, : BPC * IMW] = sl[:, :IMW].ravel()
        in_maps.append({"xh": xh, "xl": xl, **wmap})
    res = run_bass_kernel_spmd(
        nc, in_maps, list(range(NCORES)), trace=_trace
    )
    out = np.empty((NCORES * BPC, 10), np.float32)
    for c in range(NCORES):
        out[c * BPC : (c + 1) * BPC] = res.results[c]["y"].T
    if _trace:
        return out, res
    return out


# revision 44
# speedup vs baseline: 1.0985x; 1.0234x over previous
"""Trainium2 Bass kernel for nn_ExampleBinaryNet (binarized LeNet-style CNN).

Data parallel over 8 NeuronCores, 256 images each. Per core:
  conv1 (3->100, 5x5): im2col to K=75 rows (r = ky*15 + ci*5 + kx), built by
    ONE 4D-AP dma_start per 16-image tile (hi fp16 on the sync HWDGE ring,
    fp8 residual lo on the scalar ring) so the transfer spans all 75 dest
    partitions and engages ~11-16 SDMA engines. Two accumulating matmuls per
    half-image: fp16 hi + plain-fp8 lo (x = fp16(x) + 2^-6 fp8((x-hi)*64)).
  epilogue (hardtanh+maxpool folded): activations are stored centered,
    r = clip(z,-1,1) itself, via r = min(relu(z+1),2)-1; so conv weights
    stay plain signs and biases need no sign-sum corrections.
    Route D images: ACT evicts relu(P+b+1) with x-parity-deinterleaved
    write, then two fp16 tensor_tensor max ops (2x DVE mode) do the 2x2
    maxpool; route A images: DVE tensor_reduce(max) straight from PSUM then
    a tiny ACT. One batched tensor_scalar (min 2, sub 1) per tile finishes.
  conv2 (100->16, 5x5): 25 accumulating tap matmuls, K=100, 4-way PE column
    tiling into ONE shared single-bank PSUM tile; pool2 epilogue runs
    full-partition-span ops covering all 4 groups at once.
  fc1/fc2/fc3: fp16 matmuls (fc1 as 25 accumulating K=16 taps), N=256.
"""

import os
import sys

for _p in ("/opt/trn_rl_repo", "/root/.axon_site/_ro/trn_rl_repo"):
    if os.path.isdir(_p) and _p not in sys.path:
        sys.path.insert(0, _p)

import numpy as np
import ml_dtypes

import concourse.bass as bass
import concourse.tile as tile
from concourse import bacc, mybir
from concourse.bass_utils import run_bass_kernel_spmd

F32 = mybir.dt.float32
FP16 = mybir.dt.float16
FP8 = mybir.dt.float8e4
FP8NP = ml_dtypes.float8_e4m3

NCORES = 8
BPC = 256          # batch per core
NB = 16            # images per batch-tile
NT = BPC // NB     # batch-tiles per core
XPAD = BPC * 1024 + 1024   # flat padded per-channel image stream
# per-image trimmed stream: conv1 only reads offsets 0..895 of each 1024
# (y rows 28..31 are pool margin); host repacks images at 896 stride to cut
# im2col DMA bytes by 12.5%
IMW = 896
XPADT = BPC * IMW + 1024
N_ROUTE_A = 0      # images per tile pooled by DVE straight from PSUM
LO_SCALE = 64.0
# partition base for im2col/weight tiles. Would love 32 to spread the
# im2col DMA over all 16 SDMA engines, but LDWEIGHTS at base 32 may only
# span 32 partitions (BIR verifier), so K=75 weights must sit at base 0.
PBASE = 0


def _build(route_a=N_ROUTE_A, pbase=PBASE, debug=False):
    nc = bacc.Bacc("TRN2", target_bir_lowering=False, debug=False)

    # ---------------- DRAM I/O ----------------
    # xh/xl are host-pre-expanded along ky only: row a = ci*5 + ky holds the
    # channel stream shifted by 32*ky. The kx shifts come from a 3D DMA
    # access pattern [[XPAD,15],[1,5],[1,N]], giving the full 75-row im2col
    # (r = ci*25 + ky*5 + kx) per tile in one dma_start. 75 dest rows ->
    # 15 SDMA engines (5 rows each), while HBM reads stay inside a hot
    # 7.9 MB region (kx re-reads hit open rows).
    xh_d = nc.dram_tensor("xh", [15, XPADT], FP16, kind="ExternalInput")
    xl_d = nc.dram_tensor("xl", [15, XPADT], FP8, kind="ExternalInput")
    w1_d = nc.dram_tensor("w1t", [75, 112], FP16, kind="ExternalInput")
    w1l_d = nc.dram_tensor("w1l", [75, 112], FP8, kind="ExternalInput")
    w2_d = nc.dram_tensor("w2t", [100, 25, 16], FP16, kind="ExternalInput")
    w3_d = nc.dram_tensor("w3t", [16, 25, 120], FP16, kind="ExternalInput")
    w4_d = nc.dram_tensor("w4t", [120, 84], FP16, kind="ExternalInput")
    w5_d = nc.dram_tensor("w5t", [84, 10], FP16, kind="ExternalInput")
    b1p_d = nc.dram_tensor("b1p", [112, 1], F32, kind="ExternalInput")
    b2p_d = nc.dram_tensor("b2p", [112, 1], F32, kind="ExternalInput")
    b3p_d = nc.dram_tensor("b3p", [120, 1], F32, kind="ExternalInput")
    b4p_d = nc.dram_tensor("b4p", [84, 1], F32, kind="ExternalInput")
    b5e_d = nc.dram_tensor("b5e", [10, 1], F32, kind="ExternalInput")
    y_d = nc.dram_tensor("y", [10, BPC], F32, kind="ExternalOutput")
    if debug:
        dbg_r2 = nc.dram_tensor("dbg_r2", [100, NB, 196], FP16,
                                kind="ExternalOutput")
        dbg_r2p = nc.dram_tensor("dbg_r2p", [16, BPC, 25], FP16,
                                 kind="ExternalOutput")

    nA = route_a
    pb = pbase
    NJ = NB // 4  # images per conv2 column group
    # route is assigned per image-PAIR (route-D TT-maxes batch two images);
    # spread route-A pairs across the tile so DVE/ACT load interleaves
    npair = NB // 2
    na_pairs = nA // 2
    a_pairs = (
        {round(i * npair / na_pairs) for i in range(na_pairs)}
        if na_pairs > 0 else set()
    )

    with tile.TileContext(nc) as tc:
        with (
            tc.tile_pool(name="consts", bufs=1) as consts,
            tc.tile_pool(name="im_p", bufs=3) as im_p,
            tc.tile_pool(name="iml_p", bufs=2) as iml_p,
            tc.tile_pool(name="ep_p", bufs=2) as ep_p,
            tc.tile_pool(name="r2_p", bufs=2) as r2_p,
            tc.tile_pool(name="p2_p", bufs=2) as p2_p,
            tc.tile_pool(name="fc_p", bufs=1) as fc_p,
            tc.tile_pool(name="ps1_p", bufs=3, space="PSUM") as ps1_p,
            tc.tile_pool(name="ps2_p", bufs=2, space="PSUM") as ps2_p,
        ):
            # ---------------- constants ----------------
            w1full = consts.tile([pb + 75, 112], FP16, name="w1full")
            w1lfull = consts.tile([pb + 75, 112], FP8, name="w1lfull")
            w1sb = w1full[pb : pb + 75, :]
            w1lsb = w1lfull[pb : pb + 75, :]
            w2sb = consts.tile([100, 25, 16], FP16)
            w3sb = consts.tile([16, 25, 120], FP16)
            w4sb = consts.tile([120, 84], FP16)
            w5sb = consts.tile([84, 10], FP16)
            b1p = consts.tile([112, 1], F32)
            b2p = consts.tile([112, 1], F32)
            b3p = consts.tile([120, 1], F32)
            b4p = consts.tile([84, 1], F32)
            b5e = consts.tile([10, 1], F32)
            r2p = consts.tile([16, BPC, 25], FP16, name="r2p")
            # only conv1's weights/bias gate the first matmul; the rest load
            # behind the first im2col DMA to shorten the startup ramp
            for t_sb, t_d in [(w1sb, w1_d), (w1lsb, w1l_d), (b1p, b1p_d)]:
                nc.sync.dma_start(out=t_sb, in_=t_d[:])

            def load_late_consts():
                for t_sb, t_d in [
                    (w2sb, w2_d), (w3sb, w3_d), (w4sb, w4_d), (w5sb, w5_d),
                    (b2p, b2p_d), (b3p, b3p_d), (b4p, b4p_d), (b5e, b5e_d),
                ]:
                    nc.sync.dma_start(out=t_sb, in_=t_d[:])

            prev = None  # state for conv2 stage of previous batch-tile

            def conv2_block(pv):
                """conv2 + pool2 + fc-input epilogue for one batch-tile."""
                it, r2 = pv
                pg = ps2_p.tile([128, 512], F32, name=f"pg_{it}", tag="pg")
                # r2 viewed as [100, j, g, 14, 14] with local image b = 4j+g
                r2v = r2[:].rearrange("p (j g) (y x) -> p j g y x", g=4, x=14)
                for t in range(25):
                    ky, kx = divmod(t, 5)
                    for g in range(4):
                        rhs = r2v[:, :, g, ky : ky + 10, kx : kx + 10]
                        nc.tensor.matmul(
                            pg[32 * g : 32 * g + 16, 0 : 100 * NJ],
                            w2sb[:, t, :],
                            rhs,
                            start=(t == 0),
                            stop=(t == 24),
                            tile_position=(0, 32 * g),
                        )
                # one full-partition-span contiguous evict: relu(P2+(b2+1));
                # garbage partition strips (16..31 etc) are written but never
                # read downstream.
                ev2 = p2_p.tile([112, NJ, 10, 10], FP16, name=f"ev2_{it}",
                                tag="ev2")
                nc.scalar.activation(
                    out=ev2[:].rearrange("p j y x -> p (j y x)"),
                    in_=pg[0:112, 0 : 100 * NJ],
                    func=mybir.ActivationFunctionType.Relu,
                    bias=b2p[:],
                    scale=1.0,
                )
                # maxpool 2x2: x-pairs (strided, 1x) then y-pairs
                m1p = p2_p.tile([112, NJ, 10, 5], FP16, name=f"m1p_{it}",
                                tag="m1p")
                ev2v = ev2[:].rearrange("p j y (xa xb) -> p j y xa xb", xb=2)
                nc.vector.tensor_tensor(
                    m1p[:].rearrange("p j y xa -> p (j y xa)"),
                    ev2v[:, :, :, :, 0].rearrange("p j y xa -> p (j y xa)"),
                    ev2v[:, :, :, :, 1].rearrange("p j y xa -> p (j y xa)"),
                    mybir.AluOpType.max,
                )
                m2p = p2_p.tile([112, NJ, 5, 5], FP16, name=f"m2p_{it}",
                                tag="m2p")
                m1v = m1p[:].rearrange("p j (ya yb) xa -> p j ya yb xa", yb=2)
                nc.vector.tensor_tensor(
                    m2p[:], m1v[:, :, :, 0, :], m1v[:, :, :, 1, :],
                    mybir.AluOpType.max,
                )
                # r2p = min(m2, 2) - 1  (store h2 in [-1,1]); per-group
                # cross-partition remap into partitions 0..16
                r2pv = r2p[:].rearrange("p (t j g) f -> p t j g f", t=NT, g=4)
                for g in range(4):
                    nc.vector.tensor_scalar(
                        out=r2pv[:, it, :, g, :],
                        in0=m2p[32 * g : 32 * g + 16].rearrange(
                            "p j a b -> p j (a b)"
                        ),
                        scalar1=2.0,
                        scalar2=1.0,
                        op0=mybir.AluOpType.min,
                        op1=mybir.AluOpType.subtract,
                    )

            for it in range(NT):
                # -------- im2col: one rectangular dma_start each ----------
                # two half-tiles per batch-tile so conv1 of images 0-7 can
                # start while images 8-15 still stream in. Single ring: a
                # 75-row DMA already spans ~15 of the 16 physical SDMA
                # engines; a second ring only time-shares them at half rate
                # (measured 13.7 vs 27 GB/s per slot).
                HB = NB // 2
                ims, imls = [], []
                for hh in range(2):
                    base = (it * NB + hh * HB) * IMW
                    imh = im_p.tile([pb + 75, HB * IMW], FP16,
                                    name=f"im_{it}_{hh}", tag=f"im{hh}")
                    imlh = iml_p.tile([pb + 75, HB * IMW], FP8,
                                      name=f"iml_{it}_{hh}", tag=f"iml{hh}")
                    src_hi = bass.AP(
                        tensor=xh_d.ap().tensor,
                        offset=base,
                        ap=[[XPADT, 15], [1, 5], [1, HB * IMW]],
                    )
                    nc.sync.dma_start(out=imh[pb : pb + 75, :], in_=src_hi)
                    src_lo = bass.AP(
                        tensor=xl_d.ap().tensor,
                        offset=base,
                        ap=[[XPADT, 15], [1, 5], [1, HB * IMW]],
                    )
                    nc.sync.dma_start(out=imlh[pb : pb + 75, :], in_=src_lo)
                    ims.append(imh)
                    imls.append(imlh)
                    if it == 0 and hh == 0:
                        load_late_consts()

                # ---------------- conv2 of previous tile ----------------
                # emitted BEFORE this tile's conv1 so the 25 taps run dense
                # on the PE and the 4 column-tiled matmuls per tap actually
                # overlap (interleaving conv1 between them serializes all)
                if prev is not None:
                    conv2_block(prev)

                # ---------------- conv1 + pool1 epilogue ----------------
                r2u = ep_p.tile([100, NB, 196], FP16, name=f"r2u_{it}",
                                tag="r2u")
                r2 = r2_p.tile([100, NB, 196], FP16, name=f"r2_{it}", tag="r2")

                for pi in range(npair):
                    # pair the hi (then lo) matmuls of two images so the PE
                    # keeps the same stationary weights across 4 consecutive
                    # matmuls instead of swapping hi/lo every matmul
                    pair = (2 * pi, 2 * pi + 1)
                    ps1s = {}
                    for bb in pair:
                        ps1s[bb] = ps1_p.tile(
                            [112, 2, 512], F32, name=f"ps1_{it}_{bb}",
                            tag="ps1"
                        )
                    hh = (2 * pi) // (NB // 2)
                    for w_sb, i_t, first in ((w1sb, ims[hh], True),
                                             (w1lsb, imls[hh], False)):
                        for bb in pair:
                            lb = bb - hh * (NB // 2)
                            ib = i_t[
                                pb : pb + 75,
                                lb * IMW : lb * IMW + IMW,
                            ].rearrange("p (y w) -> p y w", w=32)
                            for h in range(2):
                                nc.tensor.matmul(
                                    ps1s[bb][:, h, 0:392],
                                    w_sb,
                                    ib[:, 14 * h : 14 * h + 14, 0:28],
                                    start=first,
                                    stop=not first,
                                )
                    if pi in a_pairs:
                        # route A: DVE maxpool straight from PSUM (per bank),
                        # then ACT relu(. + b1 + 1)
                        for b in pair:
                            praw = ep_p.tile([100, 2, 7, 14], FP16,
                                             name=f"praw_{it}_{b}",
                                             tag="praw")
                            for h in range(2):
                                nc.vector.tensor_reduce(
                                    out=praw[:, h],
                                    in_=ps1s[b][0:100, h, 0:392].rearrange(
                                        "p (y a x b) -> p y x a b",
                                        y=7, a=2, b=2
                                    ),
                                    axis=mybir.AxisListType.XY,
                                    op=mybir.AluOpType.max,
                                )
                            nc.scalar.activation(
                                out=r2u[:, b, :],
                                in_=praw[:].rearrange("p h y x -> p (h y x)"),
                                func=mybir.ActivationFunctionType.Relu,
                                bias=b1p[0:100],
                                scale=1.0,
                            )
                    else:
                        # route D: per-image ACT evict relu(P + b1 + 1) with
                        # x-parity deinterleave via a write AP whose inner
                        # run is 14 contiguous elements; then ONE 2x-mode
                        # TT-max per pool stage covering both images
                        ev1 = ep_p.tile([100, 2, 2, 28, 14], FP16,
                                        name=f"ev1_{it}_{pi}", tag="ev1")
                        for i, b in enumerate(pair):
                            out_ap = ev1[:, i].rearrange(
                                "p xb (h y) xa -> p xb h y xa", h=2
                            )
                            in_ap = ps1s[b][0:100, :, 0:392].rearrange(
                                "p h (y xa xb) -> p xb h y xa", y=14, xb=2
                            )
                            nc.scalar.activation(
                                out=out_ap, in_=in_ap,
                                func=mybir.ActivationFunctionType.Relu,
                                bias=b1p[0:100],
                                scale=1.0,
                            )
                        m1 = ep_p.tile([100, 2, 28, 14], FP16,
                                       name=f"m1_{it}_{pi}", tag="m1")
                        nc.vector.tensor_tensor(
                            m1[:],
                            ev1[:, :, 0],
                            ev1[:, :, 1],
                            mybir.AluOpType.max,
                        )
                        m1v = m1[:].rearrange(
                            "p i (ya yb) x -> p i ya yb x", yb=2
                        )
                        nc.vector.tensor_tensor(
                            r2u[:, 2 * pi : 2 * pi + 2, :].rearrange(
                                "p i (y x) -> p i y x", x=14
                            ),
                            m1v[:, :, :, 0, :],
                            m1v[:, :, :, 1, :],
                            mybir.AluOpType.max,
                        )

                # batched: r2 = min(r2u, 2) - 1  (store h1 in [-1,1])
                nc.vector.tensor_scalar(
                    out=r2[:].rearrange("p b f -> p (b f)"),
                    in0=r2u[:].rearrange("p b f -> p (b f)"),
                    scalar1=2.0,
                    scalar2=1.0,
                    op0=mybir.AluOpType.min,
                    op1=mybir.AluOpType.subtract,
                )

                if debug and it == 0:
                    nc.sync.dma_start(out=dbg_r2[:], in_=r2[:])

                prev = (it, r2)

            conv2_block(prev)

            # ---------------- fully connected layers ----------------
            if debug:
                nc.sync.dma_start(out=dbg_r2p[:], in_=r2p[:])
            ps3 = ps1_p.tile([120, BPC], F32, name="ps3", tag="ps1")
            for p in range(25):
                nc.tensor.matmul(
                    ps3[:],
                    w3sb[:, p, :],
                    r2p[:, :, p],
                    start=(p == 0),
                    stop=(p == 24),
                )
            u3 = fc_p.tile([120, BPC], F32)
            nc.scalar.activation(
                out=u3[:], in_=ps3[:],
                func=mybir.ActivationFunctionType.Relu,
                bias=b3p[:], scale=1.0,
            )
            r3 = fc_p.tile([120, BPC], FP16)
            nc.vector.tensor_scalar(
                out=r3[:], in0=u3[:], scalar1=2.0, scalar2=1.0,
                op0=mybir.AluOpType.min, op1=mybir.AluOpType.subtract,
            )

            ps4 = ps1_p.tile([84, BPC], F32, name="ps4", tag="ps1")
            nc.tensor.matmul(ps4[:], w4sb[:], r3[:], start=True, stop=True)
            u4 = fc_p.tile([84, BPC], F32)
            nc.scalar.activation(
                out=u4[:], in_=ps4[:],
                func=mybir.ActivationFunctionType.Relu,
                bias=b4p[:], scale=1.0,
            )
            r4 = fc_p.tile([84, BPC], FP16)
            nc.vector.tensor_scalar(
                out=r4[:], in0=u4[:], scalar1=2.0, scalar2=1.0,
                op0=mybir.AluOpType.min, op1=mybir.AluOpType.subtract,
            )

            ps5 = ps1_p.tile([10, BPC], F32, name="ps5", tag="ps1")
            nc.tensor.matmul(ps5[:], w5sb[:], r4[:], start=True, stop=True)
            y_sb = fc_p.tile([10, BPC], F32)
            nc.vector.tensor_scalar_add(y_sb[:], ps5[:], b5e[:])
            nc.sync.dma_start(out=y_d[:], in_=y_sb[:])

    nc.compile()
    return nc


_NC_CACHE = {}


def _get_nc(route_a=N_ROUTE_A, pbase=PBASE, debug=False):
    key = (route_a, pbase, debug)
    if key not in _NC_CACHE:
        _NC_CACHE[key] = _build(route_a, pbase, debug)
    return _NC_CACHE[key]


def _prep_weights(w1, b1, w2, b2, w3, b3, w4, b4, w5, b5):
    s1 = np.sign(w1).astype(np.float32)  # [100,3,5,5]
    s2 = np.sign(w2).astype(np.float32)  # [16,100,5,5]
    s3 = np.sign(w3).astype(np.float32)  # [120,400]
    s4 = np.sign(w4).astype(np.float32)  # [84,120]
    s5 = np.sign(w5).astype(np.float32)  # [10,84]

    # conv1 lhsT rows: r = ci*25 + ky*5 + kx; cols padded 100 -> 112
    w1t = np.zeros((75, 112), np.float32)
    w1t[:, :100] = s1.transpose(1, 2, 3, 0).reshape(75, 100)
    w1l = w1t / LO_SCALE
    # conv2 lhsT: [ci, t=ky*5+kx, o] (plain signs; rhs is centered h1)
    w2t = np.ascontiguousarray(
        s2.transpose(1, 2, 3, 0).reshape(100, 25, 16)
    ).astype(np.float16)
    # fc1 taps: [c2, p, o] = s3[o, c2*25+p]
    w3t = np.ascontiguousarray(
        s3.reshape(120, 16, 25).transpose(1, 2, 0)
    ).astype(np.float16)
    w4t = np.ascontiguousarray(s4.T).astype(np.float16)
    w5t = np.ascontiguousarray(s5.T).astype(np.float16)

    def colvec(v, n):
        out = np.zeros((n, 1), np.float32)
        out[: len(v), 0] = v
        return out

    b1p = colvec(b1 + 1.0, 112)
    # b2 replicated across the 4 conv2 column-group partition strips
    b2p = np.zeros((112, 1), np.float32)
    for g in range(4):
        b2p[32 * g : 32 * g + 16, 0] = b2 + 1.0
    b3p = colvec(b3 + 1.0, 120)
    b4p = colvec(b4 + 1.0, 84)
    b5e = colvec(b5, 10)
    return {
        "w1t": w1t.astype(np.float16), "w1l": w1l.astype(FP8NP),
        "w2t": w2t, "w3t": w3t, "w4t": w4t, "w5t": w5t,
        "b1p": b1p, "b2p": b2p, "b3p": b3p, "b4p": b4p, "b5e": b5e,
    }


def kernel(x, w1, b1, w2, b2, w3, b3, w4, b4, w5, b5, _trace=False,
           _route_a=N_ROUTE_A, _pbase=PBASE, _debug=False):
    x = np.asarray(x, dtype=np.float32)
    wmap = _prep_weights(
        np.asarray(w1), np.asarray(b1), np.asarray(w2), np.asarray(b2),
        np.asarray(w3), np.asarray(b3), np.asarray(w4), np.asarray(b4),
        np.asarray(w5), np.asarray(b5),
    )
    nc = _get_nc(_route_a, _pbase, _debug)
    in_maps = []
    for c in range(NCORES):
        xs = x[c * BPC : (c + 1) * BPC]  # [256,3,32,32]
        xs = np.ascontiguousarray(
            xs.transpose(1, 0, 2, 3).reshape(3, BPC * 1024)
        )
        xh0 = np.zeros((3, XPAD), np.float16)
        xh0[:, : BPC * 1024] = xs.astype(np.float16)
        xl0 = np.zeros((3, XPAD), FP8NP)
        xl0[:, : BPC * 1024] = (
            (xs - xh0[:, : BPC * 1024].astype(np.float32)) * LO_SCALE
        ).astype(FP8NP)
        # pre-expand along ky (shift 32*ky) and trim each image's stream to
        # IMW=896 elements (rows y>=28 are never read by conv1)
        xh = np.zeros((15, XPADT), np.float16)
        xl = np.zeros((15, XPADT), FP8NP)
        for ci in range(3):
            for ky in range(5):
                s = 32 * ky
                a = ci * 5 + ky
                sh = xh0[ci, s : s + BPC * 1024].reshape(BPC, 1024)
                sl = xl0[ci, s : s + BPC * 1024].reshape(BPC, 1024)
                xh[a, : BPC * IMW] = sh[:, :IMW].ravel()
                xl[a, : BPC * IMW] = sl[:, :IMW].ravel()
        in_maps.append({"xh": xh, "xl": xl, **wmap})
    res = run_bass_kernel_spmd(
        nc, in_maps, list(range(NCORES)), trace=_trace
    )
    out = np.empty((NCORES * BPC, 10), np.float32)
    for c in range(NCORES):
        out[c * BPC : (c + 1) * BPC] = res.results[c]["y"].T
    if _trace:
        return out, res
    return out
